# revision 38
# baseline (speedup 1.0000x reference)
import sys

if '/opt/trn_rl_repo' not in sys.path:
    sys.path.insert(0, '/opt/trn_rl_repo')

import numpy as np

B, D, Q, H = 16, 1024, 128, 1024
NCORES = 8
NB = B // NCORES
NT = D // 128
HHALF = 512

_CACHE = {}

UDT_EVAC = "ADDADDDA"
UQT_EVAC = "D"
ENP_EVAC = "DDDDDDDD"
AD_ENG = {0: "AADAADAA", 1: "AADAADAA"}
C3_ENG = {0: "DDGDDGDD", 1: "DGGDGDGD"}
C4_ENG = {0: "DGMDDGMD",
          1: "DMGDDGMD"}


def build_nc():
    import concourse.bacc as bacc
    import concourse.tile as tile
    from concourse import mybir, masks
    import concourse.bass as bass
    from contextlib import ExitStack

    ts = bass.ts
    f32 = mybir.dt.float32
    bf16 = mybir.dt.bfloat16
    AF = mybir.ActivationFunctionType
    ALU = mybir.AluOpType

    nc = bacc.Bacc("TRN2", target_bir_lowering=False, debug=False)

    Ud_dram = nc.dram_tensor("U_d", [NB, D, H], bf16, kind="ExternalInput")
    Uq_dram = nc.dram_tensor("U_q", [NB, Q, H], bf16, kind="ExternalInput")
    w_dram = nc.dram_tensor("wc_w", [128, 3, NT], f32, kind="ExternalInput")
    mb_dram = nc.dram_tensor("d_mask", [NB, 128, NT + 1], f32,
                             kind="ExternalInput")
    V_dram = nc.dram_tensor("V", [NB, D, 4 * H], bf16, kind="ExternalOutput")

    with tile.TileContext(nc) as tc, ExitStack() as ctx:
        const = ctx.enter_context(tc.tile_pool(name="const", bufs=1))
        big = ctx.enter_context(tc.tile_pool(name="big", bufs=2))
        med = ctx.enter_context(tc.tile_pool(name="med", bufs=2))
        vec = ctx.enter_context(tc.tile_pool(name="vec", bufs=2))
        outp = ctx.enter_context(tc.tile_pool(name="outp", bufs=8))
        ps_pp = ctx.enter_context(tc.tile_pool(name="ps_pp", bufs=3, space="PSUM"))
        ps_sd = ctx.enter_context(tc.tile_pool(name="ps_sd", bufs=1, space="PSUM"))

        w_cols = const.tile([128, 3, NT], f32, tag="wcols")
        nc.gpsimd.dma_start(w_cols[:], w_dram[:])
        wd16 = const.tile([128, NT], bf16, tag="wd16")
        wq16 = const.tile([128, NT], bf16, tag="wq16")
        nc.vector.tensor_copy(wd16[:], w_cols[:, 0, :])
        nc.vector.tensor_copy(wq16[:], w_cols[:, 1, :])
        ident16 = const.tile([128, 128], bf16, tag="id16")
        masks.make_identity(nc, ident16[:])
        ones16 = const.tile([128, 1], bf16, tag="ones16")
        nc.vector.memset(ones16[:], 1.0)

        st = {}
        for b in range(NB):
            s = st[b] = {}
            Ud = s['Ud'] = big.tile([128, NT, H], bf16, tag="Ud", name=f"Ud{b}")
            Ud_src = Ud_dram[b].rearrange("(t p) h -> p t h", p=128)
            if b == 0:
                for q2 in range(2):
                    nc.sync.dma_start(Ud[:, 0, ts(q2, 512)],
                                      Ud_src[:, 0, ts(q2, 512)])
                for t in range(1, NT):
                    nc.sync.dma_start(Ud[:, t, :], Ud_src[:, t, :])
            else:
                nc.sync.dma_start(Ud[:, 0:4, :], Ud_src[:, 0:4, :])
                st[b]['dma_late'] = (Ud[:, 4:NT, :], Ud_src[:, 4:NT, :])
            s['Uq16'] = med.tile([128, H], bf16, tag="Uq16", name=f"Uq16_{b}")
            nc.gpsimd.dma_start(s['Uq16'][:], Uq_dram[b])
            mk = s['mk'] = vec.tile([128, NT + 1], f32, tag="mk", name=f"mk{b}")
            nc.sync.dma_start(mk[:], mb_dram[b])

        def udsec(b, i8):
            rows = slice(i8 * 128, (i8 + 1) * 128)
            nc.sync.dma_start(V_dram[b, rows, 0:H], Ud_dram[b, rows, :])


        def ab_setup(b):
            s = st[b]
            s['UdT'] = big.tile([128, NT, D], bf16, tag="UdT", name=f"UdT{b}")
            s['UqT'] = med.tile([128, NT, Q], bf16, tag="UqT", name=f"UqT{b}")
            s['YT'] = med.tile([128, NT, Q], bf16, tag="YT", name=f"YT{b}")
            s['ET'] = med.tile([128, D], bf16, tag="ET", name=f"ET{b}")
            s['rinv'] = vec.tile([128, NT], f32, tag="rinv", name=f"rinv{b}")
            s['smA'] = ps_sd.tile([128, 2 * NT + 1], f32, tag="smA",
                                  name=f"smA{b}")
            s['sqb'] = vec.tile([128, 1], f32, tag="sqb", name=f"sqb{b}")
            s['ST'] = [None, None]

        def ab_uq(b):
            s = st[b]
            Uq16, UqT, YT, mk = s['Uq16'], s['UqT'], s['YT'], s['mk']
            tq = ps_pp.tile([128, NT * 128], bf16, tag="pp", name=f"tq{b}",
                            padded_shape=[128, 2048])
            for k in range(NT):
                nc.tensor.transpose(tq[:, ts(k, 128)],
                                    Uq16[:, ts(k, 128)], ident16[:])
            if UQT_EVAC[0] == 'A':
                nc.scalar.copy(UqT[:], tq[:])
            else:
                nc.vector.tensor_copy(UqT[:], tq[:])
            for k in range(NT):
                nc.vector.tensor_scalar_mul(YT[:, k, :], UqT[:, k, :],
                                            w_cols[:, 2, k:k + 1])
            sqc_ps = s['smA'][:, 2 * NT:2 * NT + 1]
            for k in range(NT):
                nc.tensor.matmul(sqc_ps[:], UqT[:, k, :], wq16[:, k:k + 1],
                                 start=(k == 0), stop=(k == NT - 1))
            nc.scalar.activation(s['sqb'][:], sqc_ps[:], AF.Identity,
                                 bias=mk[:, 0:1])

        def ab_half(b, hf):
            s = st[b]
            Ud, UdT, YT, ET = s['Ud'], s['UdT'], s['YT'], s['ET']
            for t in range(4 * hf, 4 * hf + 4):
                tp = ps_pp.tile([128, NT * 128], bf16, tag="pp",
                                name=f"tp{b}_{t}", padded_shape=[128, 2048])
                for k in range(NT):
                    nc.tensor.transpose(tp[:, ts(k, 128)],
                                        Ud[:, t, ts(k, 128)], ident16[:])
                dst = UdT[:, :, ts(t, 128)]
                if UDT_EVAC[t] == 'A':
                    nc.scalar.copy(dst, tp[:])
                else:
                    nc.vector.tensor_copy(dst, tp[:])
            STh = ps_pp.tile([128, HHALF], f32, tag="pp", name=f"ST{b}_{hf}",
                             padded_shape=[128, 1024])
            s['ST'][hf] = STh
            for k in range(NT):
                nc.tensor.matmul(STh[:], YT[:, k, :],
                                 UdT[:, k, ts(hf, HHALF)],
                                 start=(k == 0), stop=(k == NT - 1))
            nc.scalar.activation(ET[:, ts(hf, HHALF)], STh[:], AF.Exp,
                                 bias=s['sqb'][:])
            rc_ps = s['smA'][:, NT:2 * NT]
            for dc in range(4 * hf, 4 * hf + 4):
                nc.tensor.matmul(rc_ps[:, dc:dc + 1], ET[:, ts(dc, 128)],
                                 ones16[:], start=True, stop=True)
            nc.vector.reciprocal(s['rinv'][:, 4 * hf:4 * hf + 4],
                                 rc_ps[:, 4 * hf:4 * hf + 4])

        def ab_sd(b):
            s = st[b]
            UdT, mk = s['UdT'], s['mk']
            sdc_ps = s['smA'][:, 0:NT]
            for dblk in range(NT):
                for k in range(NT):
                    nc.tensor.matmul(sdc_ps[:, dblk:dblk + 1],
                                     UdT[:, k, ts(dblk, 128)], wd16[:, k:k + 1],
                                     start=(k == 0), stop=(k == NT - 1))
            sd_sum = vec.tile([128, NT], f32, tag="sdsum", name=f"sds{b}")
            nc.vector.tensor_add(sd_sum[:], sdc_ps[:], mk[:, 1:NT + 1])
            exps = s['exps'] = vec.tile([128, NT], f32, tag="exps",
                                        name=f"exps{b}")
            nc.scalar.activation(exps[:], sd_sum[:], AF.Exp)

        def stage_AB(b):
            ab_setup(b)
            s = st[b]
            Ud, UdT, UqT, YT = s['Ud'], s['UdT'], s['UqT'], s['YT']
            Uq16, mk, ET = s['Uq16'], s['mk'], s['ET']
            for t in range(NT):
                tp = ps_pp.tile([128, NT * 128], bf16, tag="pp",
                                name=f"tp{b}_{t}", padded_shape=[128, 2048])
                for k in range(NT):
                    nc.tensor.transpose(tp[:, ts(k, 128)],
                                        Ud[:, t, ts(k, 128)], ident16[:])
                dst = UdT[:, :, ts(t, 128)]
                if UDT_EVAC[t] == 'A':
                    nc.scalar.copy(dst, tp[:])
                else:
                    nc.vector.tensor_copy(dst, tp[:])
            tq = ps_pp.tile([128, NT * 128], bf16, tag="pp", name=f"tq{b}",
                            padded_shape=[128, 2048])
            for k in range(NT):
                nc.tensor.transpose(tq[:, ts(k, 128)],
                                    Uq16[:, ts(k, 128)], ident16[:])
            if UQT_EVAC[0] == 'A':
                nc.scalar.copy(UqT[:], tq[:])
            else:
                nc.vector.tensor_copy(UqT[:], tq[:])
            for k in range(NT):
                nc.vector.tensor_scalar_mul(YT[:, k, :], UqT[:, k, :],
                                            w_cols[:, 2, k:k + 1])
            ST = ps_pp.tile([128, D], f32, tag="pp", name=f"ST{b}")
            for hf in range(2):
                for k in range(NT):
                    nc.tensor.matmul(ST[:, ts(hf, HHALF)], YT[:, k, :],
                                     UdT[:, k, ts(hf, HHALF)],
                                     start=(k == 0), stop=(k == NT - 1))
            smA = s['smA']
            sdc_ps, rc_ps = smA[:, 0:NT], smA[:, NT:2 * NT]
            sqc_ps = smA[:, 2 * NT:2 * NT + 1]
            for dblk in range(NT):
                for k in range(NT):
                    nc.tensor.matmul(sdc_ps[:, dblk:dblk + 1],
                                     UdT[:, k, ts(dblk, 128)], wd16[:, k:k + 1],
                                     start=(k == 0), stop=(k == NT - 1))
            for k in range(NT):
                nc.tensor.matmul(sqc_ps[:], UqT[:, k, :], wq16[:, k:k + 1],
                                 start=(k == 0), stop=(k == NT - 1))
            nc.scalar.activation(s['sqb'][:], sqc_ps[:], AF.Identity,
                                 bias=mk[:, 0:1])
            sd_sum = vec.tile([128, NT], f32, tag="sdsum", name=f"sds{b}")
            nc.vector.tensor_add(sd_sum[:], sdc_ps[:], mk[:, 1:NT + 1])
            exps = s['exps'] = vec.tile([128, NT], f32, tag="exps",
                                        name=f"exps{b}")
            nc.scalar.activation(exps[:], sd_sum[:], AF.Exp)
            for hf in range(2):
                nc.scalar.activation(ET[:, ts(hf, HHALF)], ST[:, ts(hf, HHALF)],
                                     AF.Exp, bias=s['sqb'][:])
            for dc in range(NT):
                nc.tensor.matmul(rc_ps[:, dc:dc + 1], ET[:, ts(dc, 128)],
                                 ones16[:], start=True, stop=True)
            nc.vector.reciprocal(s['rinv'][:], rc_ps[:])

        def stage_E1(b, fill=(), dcs=tuple(range(NT))):
            s = st[b]
            Ud, Uq16, ET, rinv = s['Ud'], s['Uq16'], s['ET'], s['rinv']
            out2s = {}
            for dc in dcs:
                lhs = ET[:, ts(dc, 128)]
                rdc = rinv[:, dc:dc + 1]
                out2 = outp.tile([128, 2, H], bf16, tag="out2",
                                 name=f"out2_{b}_{dc}")
                out2s[dc] = out2
                a_ps = ps_pp.tile([128, H], f32, tag="pp",
                                  name=f"aps{b}_{dc}")
                for hf in range(2):
                    nc.tensor.matmul(a_ps[:, ts(hf, HHALF)], lhs,
                                     Uq16[:, ts(hf, HHALF)],
                                     start=True, stop=True)
                if AD_ENG[b][dc] == 'A':
                    nc.scalar.mul(out2[:, 0, :], a_ps[:], rdc)
                else:
                    nc.vector.tensor_scalar_mul(out2[:, 0, :], a_ps[:], rdc)
            order = ([dc for dc in dcs if C3_ENG[b][dc] != 'G'] +
                     [dc for dc in dcs if C3_ENG[b][dc] == 'G'])
            for dc in order:
                out2 = out2s[dc]
                eng = nc.gpsimd if C3_ENG[b][dc] == 'G' else nc.vector
                eng.tensor_mul(out2[:, 1, :], out2[:, 0, :], Ud[:, dc, :])
            for n, dc in enumerate(order):
                rows = slice(dc * 128, (dc + 1) * 128)
                nc.sync.dma_start(V_dram[b, rows, H:3 * H], out2s[dc][:])
                for fb, fi in dict(fill).get(n, ()):
                    udsec(fb, fi)

        def cd_en(b):
            s = st[b]
            ET, exps = s['ET'], s['exps']
            EN = s['EN'] = med.tile([128, NT, Q], bf16, tag="EN",
                                    name=f"EN{b}")
            te = ps_sd.tile([128, NT * 128], bf16, tag="te", name=f"te{b}")
            for ec in range(NT):
                nc.tensor.transpose(te[:, ts(ec, 128)],
                                    ET[:, ts(ec, 128)], ident16[:])
            for ec in range(NT):
                if ENP_EVAC[ec] == 'A':
                    nc.scalar.mul(EN[:, ec, :], te[:, ts(ec, 128)],
                                  exps[:, ec:ec + 1])
                else:
                    nc.vector.tensor_scalar_mul(EN[:, ec, :],
                                                te[:, ts(ec, 128)],
                                                exps[:, ec:ec + 1])

        def cd_w(b):
            s = st[b]
            Ud, EN = s['Ud'], s['EN']
            Wb = ps_pp.tile([128, H], f32, tag="pp", name=f"Wb{b}")
            for hf in range(2):
                for et in range(NT):
                    nc.tensor.matmul(Wb[:, ts(hf, HHALF)], EN[:, et, :],
                                     Ud[:, et, ts(hf, HHALF)],
                                     start=(et == 0), stop=(et == NT - 1))
            smB = ps_pp.tile([128, 1], f32, tag="pp", name=f"c2_{b}",
                             padded_shape=[128, 1024])
            for et in range(NT):
                nc.tensor.matmul(smB[:], EN[:, et, :], ones16[:],
                                 start=(et == 0), stop=(et == NT - 1))
            c2inv = vec.tile([128, 1], f32, tag="c2inv")
            nc.vector.reciprocal(c2inv[:], smB[:])
            W = s['W'] = med.tile([128, H], bf16, tag="W", name=f"W{b}")
            nc.vector.tensor_scalar_mul(W[:], Wb[:], c2inv[:])

        def stage_CD(b):
            cd_en(b)
            cd_w(b)

        def stage_E2(b, fill=(), dcs=tuple(range(NT))):
            s = st[b]
            Ud, ET, rinv, W = s['Ud'], s['ET'], s['rinv'], s['W']
            out4s, a4s = {}, {}
            for dc in dcs:
                lhs = ET[:, ts(dc, 128)]
                rdc = rinv[:, dc:dc + 1]
                out4 = outp.tile([128, H], bf16, tag="out4",
                                 name=f"out4_{b}_{dc}")
                out4s[dc] = out4
                r_ps = ps_pp.tile([128, H], f32, tag="pp",
                                  name=f"rps{b}_{dc}")
                for hf in range(2):
                    nc.tensor.matmul(r_ps[:, ts(hf, HHALF)], lhs,
                                     W[:, ts(hf, HHALF)],
                                     start=True, stop=True)
                if C4_ENG[b][dc] == 'D':
                    nc.vector.scalar_tensor_tensor(
                        out4[:], r_ps[:], rdc, Ud[:, dc, :],
                        ALU.mult, ALU.mult)
                else:
                    A4 = outp.tile([128, H], bf16, tag="A4",
                                   name=f"A4_{b}_{dc}")
                    nc.scalar.mul(A4[:], r_ps[:], rdc)
                    a4s[dc] = A4
            nD = 0
            for dc in dcs:
                if C4_ENG[b][dc] == 'D':
                    rows = slice(dc * 128, (dc + 1) * 128)
                    nc.sync.dma_start(V_dram[b, rows, 3 * H:4 * H],
                                      out4s[dc][:])
                    nD += 1
                    for fb, fi in dict(fill).get(nD - 1, ()):
                        udsec(fb, fi)
            order = ([dc for dc in dcs if C4_ENG[b][dc] == 'G'] +
                     [dc for dc in dcs if C4_ENG[b][dc] == 'M'])
            for dc in order:
                eng = nc.gpsimd if C4_ENG[b][dc] == 'G' else nc.vector
                eng.tensor_mul(out4s[dc][:], a4s[dc][:], Ud[:, dc, :])
            for n, dc in enumerate(order):
                rows = slice(dc * 128, (dc + 1) * 128)
                nc.sync.dma_start(V_dram[b, rows, 3 * H:4 * H], out4s[dc][:])
                for fb, fi in dict(fill).get(('m', n), ()):
                    udsec(fb, fi)

        stage_AB(0)
        udsec(0, 0)
        udsec(0, 1)
        nc.sync.dma_start(*st[1]['dma_late'])
        udsec(0, 2)
        stage_E1(0, fill={3: [(0, 3)]})
        stage_AB(1)
        udsec(0, 4)
        cd_en(0)
        stage_E1(1, fill={1: [(0, 5)]}, dcs=(0, 1, 2, 3))
        udsec(0, 6)
        cd_w(0)
        stage_E1(1, fill={1: [(0, 7)]}, dcs=(4, 5, 6, 7))
        udsec(1, 0)
        udsec(1, 1)
        cd_en(1)
        stage_E2(0, fill={1: [(1, 2)]}, dcs=(0, 1, 2, 3))
        udsec(1, 3)
        cd_w(1)
        stage_E2(0, fill={1: [(1, 4), (1, 5)], ('m', 0): [(1, 6), (1, 7)]},
                 dcs=(4, 5, 6, 7))
        stage_E2(1)

    nc.compile()
    return nc


def _get_nc():
    if 'nc' not in _CACHE:
        _CACHE['nc'] = build_nc()
    return _CACHE['nc']


def make_in_maps(inputs):
    import ml_dtypes
    bf16 = ml_dtypes.bfloat16
    U_d = np.asarray(inputs['U_d'], dtype=np.float32).astype(bf16)
    U_q = np.asarray(inputs['U_q'], dtype=np.float32).astype(bf16)
    wc_w = np.asarray(inputs['wc_w'], dtype=np.float32)
    q_mask = np.asarray(inputs['q_mask'], dtype=np.int32)
    d_mask = np.asarray(inputs['d_mask'], dtype=np.int32)
    w_cols = np.ascontiguousarray(
        wc_w.reshape(3, NT, 128).transpose(2, 0, 1))
    qbias = ((q_mask.astype(np.float32) - 1.0) * 30.0)[:, :, None]
    dbias = ((d_mask.astype(np.float32) - 1.0) * 30.0) \
        .reshape(B, NT, 128).transpose(0, 2, 1)
    mbias = np.ascontiguousarray(
        np.concatenate([qbias, dbias], axis=2))
    in_maps = []
    for c in range(NCORES):
        s = slice(c * NB, (c + 1) * NB)
        in_maps.append({
            'U_d': np.ascontiguousarray(U_d[s]),
            'U_q': np.ascontiguousarray(U_q[s]),
            'wc_w': w_cols,
            'd_mask': mbias[s],
        })
    return in_maps


def run(inputs, trace=False, **kw):
    from concourse.bass_utils import run_bass_kernel_spmd
    nc = _get_nc()
    res = run_bass_kernel_spmd(nc, make_in_maps(inputs), list(range(NCORES)),
                               trace=trace, **kw)
    out = np.concatenate(
        [np.asarray(res.results[c]['V']).astype(np.float32)
         for c in range(NCORES)], axis=0)
    return out, res


def kernel(**inputs) -> np.ndarray:
    out, _ = run(inputs, trace=False)
    return out
